# revision 5
# baseline (speedup 1.0000x reference)
"""Custom cross-entropy loss (CE + length/line-count penalties) on 8 trn2 cores.

v7 = v4 (bf16 streaming, contiguous-stripe layout) with the DVE max-scan
restructured around the 2x-rate bf16 tensor_tensor path:

  - level 1: tensor_max pairs adjacent 250-elem segments (bf16 2x_1P mode,
    ~0.5 cyc/elem) -> [P, nseg/2, 250]
  - level 2: fused tensor_tensor_reduce pairs those again AND max-reduces,
    emitting one max per contiguous 1000-elem block into cm (the elementwise
    output goes to a throwaway scratch).
  Winning block per row is refetched as one contiguous [P, 1000] indirect
  gather; max_index inside it preserves exact first-occurrence argmax
  semantics end-to-end (blocks are position-ordered and contiguous).

ScalarE's exp stream is the critical path; rows 0-3 run 4000-wide ACTIVATEs
and rows 4-7 run 8000-wide ones (an in-run A/B of the width-dependent
SBUF-read degradation). Row 0's first two chunks are 4000 wide to cut the
first-compute latency.
"""

import ml_dtypes
import numpy as np

import concourse.bass as bass
import concourse.bacc as bacc
import concourse.tile as tile
from concourse import mybir
from concourse import bass_utils

NEXT_LINE = 2
EOS_ID = 1
IGNORE = 0
ALPHAS = (0.98, 0.01, 0.01)

B, S, V = 4, 2048, 32000
N_CORES = 8
P = 128                      # SBUF partitions
R = (B * S) // N_CORES       # rows per core = 1024
RPP = R // P                 # rows per partition = 8
BW = 1000                    # argmax block width (refetch granularity)
NB = V // BW                 # blocks per row = 32

F32 = mybir.dt.float32
BF16 = mybir.dt.bfloat16
U32 = mybir.dt.uint32

# per-row DMA tile widths (elems); row 0 starts small to cut head latency
TILES_ROW0 = [2000, 2000, 4000, 4000, 4000, 8000, 8000]
TILES_ROW = [8000, 8000, 8000, 8000]
# 8000-wide bf16 ACTIVATEs measured 1.042 cyc/elem vs 1.084 at 4000 (and
# half the ACTIVATION_READ_ACCUMULATOR count) -> use everywhere; narrow
# leading row-0 tiles still run narrower ACTs (min(aw, tw)).
ACT_W = [8000] * 8


def build_bass(rows=R, v=V, x_bufs=7):
    rpp = rows // P
    nc = bacc.Bacc("TRN2", debug=False, num_devices=N_CORES, enable_asserts=False)

    logits = nc.dram_tensor("logits", [rows, v], BF16, kind="ExternalInput").ap()
    # rb[p, r] = (p*rpp + r) * NB   (row base into the [rows*NB, BW] table)
    rb = nc.dram_tensor("rb", [P, rpp], U32, kind="ExternalInput").ap()

    # fused output: cols [0:rpp]=sumexp, [rpp:2rpp]=block idx, [2rpp:3rpp]=widx
    o_all = nc.dram_tensor("o_all", [P, 3 * rpp], F32, kind="ExternalOutput").ap()

    xv = logits.rearrange("(p r) v -> p r v", r=rpp)          # [P, rpp, V]
    tbl = logits.rearrange("r (w u) -> (r w) u", u=BW)        # [rows*NB, BW]

    with tile.TileContext(nc) as tc:
        with (
            tc.tile_pool(name="persist", bufs=1) as pp,
            tc.tile_pool(name="xpool", bufs=x_bufs) as px,
            tc.tile_pool(name="epool", bufs=2) as pe,
            tc.tile_pool(name="gpool", bufs=2) as pg1,
            tc.tile_pool(name="spool", bufs=2) as psc,
            tc.tile_pool(name="wpool", bufs=rpp) as pw,
            tc.tile_pool(name="mpool", bufs=rpp) as pm,
            tc.tile_pool(name="stats", bufs=4) as ps,
        ):
            rb_sb = pp.tile([P, rpp], U32)
            out_sb = pp.tile([P, 3 * rpp], F32)
            ridx_all = pp.tile([P, rpp], U32)


            wins = []
            gm8s = []
            anchors = []
            row_first_anchor = []
            for r in range(rpp):
                widths = TILES_ROW0 if r == 0 else TILES_ROW
                aw = ACT_W[r]
                se = ps.tile([P, 8], F32, tag="se")
                cm = ps.tile([P, NB], F32, tag="cm")
                se_col = 0
                blk = 0
                row_first_anchor.append(len(anchors))
                for tw in widths:
                    off = blk * BW               # elem offset of tile in row
                    x = px.tile([P, tw], BF16, tag="x")
                    nc.sync.dma_start(out=x[:], in_=xv[:, r, off : off + tw])
                    if r == 0 and off == 0:
                        # rb is tiny and first needed ~30us in; issue it
                        # behind the first logits chunk, not ahead of it
                        nc.sync.dma_start(out=rb_sb[:], in_=rb[:])
                    # ScalarE: exp + fused sum per aw-wide span
                    for o in range(max(tw // aw, 1)):
                        w = min(aw, tw)
                        ex = pe.tile([P, w], BF16, tag="ex")
                        nc.scalar.activation(
                            out=ex[:, : w], in_=x[:, o * w : (o + 1) * w],
                            func=mybir.ActivationFunctionType.Exp,
                            accum_out=se[:, se_col : se_col + 1],
                        )
                        se_col += 1
                    # DVE: level-1 pairwise segment max (bf16 2x)
                    nseg = tw // 250
                    xr = x[:].rearrange("p (s two j) -> p s two j", two=2, j=250)
                    g1 = pg1.tile([P, tw // 2], BF16, tag="g1")
                    g1r = g1[:].rearrange("p (s j) -> p s j", j=250)
                    l1 = nc.vector.tensor_max(
                        out=g1r, in0=xr[:, :, 0, :], in1=xr[:, :, 1, :]
                    )
                    anchors.append(l1)
                    # level-2 pairwise max, then segmented block-max reduce
                    g1p = g1[:].rearrange("p (s two j) -> p s two j", two=2, j=250)
                    scr = psc.tile([P, tw // 4], BF16, tag="scr")
                    scrr = scr[:].rearrange("p (s j) -> p s j", j=250)
                    nc.vector.tensor_max(
                        out=scrr, in0=g1p[:, :, 0, :], in1=g1p[:, :, 1, :]
                    )
                    nc.vector.reduce_max(
                        out=cm[:, blk : blk + tw // BW],
                        in_=scrr,
                        axis=mybir.AxisListType.X,
                    )
                    blk += tw // BW
                # row wrap-up: block max, refetch, denominator
                gm8 = pm.tile([P, 8], F32, tag="gm8")
                nc.vector.max(out=gm8[:], in_=cm[:])
                gm8s.append(gm8)
                c8 = ps.tile([P, 8], U32, tag="c8")
                nc.vector.max_index(out=c8[:], in_max=gm8[:], in_values=cm[:])
                nc.vector.tensor_copy(
                    out=out_sb[:, rpp + r : rpp + r + 1], in_=c8[:, 0:1]
                )
                nc.vector.tensor_add(
                    out=ridx_all[:, r : r + 1],
                    in0=rb_sb[:, r : r + 1],
                    in1=c8[:, 0:1],
                )
                win = pw.tile([P, BW], BF16, tag="win")
                nc.gpsimd.indirect_dma_start(
                    out=win[:],
                    out_offset=None,
                    in_=tbl[:],
                    in_offset=bass.IndirectOffsetOnAxis(
                        ap=ridx_all[:, r : r + 1], axis=0
                    ),
                )
                wins.append(win)
                nc.vector.reduce_sum(
                    out=out_sb[:, r : r + 1],
                    in_=se[:, : se_col],
                    axis=mybir.AxisListType.X,
                )

            # within-block argmax of each row's winning block; anchored two
            # rows downstream so the in-order DVE never stalls on an
            # in-flight indirect gather mid-stream.
            from concourse.tile_rust import add_dep_helper

            for r in range(rpp):
                ai = (
                    row_first_anchor[r + 2]
                    if r + 2 < rpp
                    else len(anchors) - 1
                )
                anchor = anchors[ai]
                b8 = ps.tile([P, 8], BF16, tag="b8")
                cp = nc.vector.tensor_copy(
                    out=b8[:], in_=gm8s[r][:, 0:1].to_broadcast([P, 8])
                )
                add_dep_helper(cp.ins, anchor.ins, sync=False, reason="defer-winidx")
                w8 = ps.tile([P, 8], U32, tag="w8")
                nc.vector.max_index(out=w8[:], in_max=b8[:], in_values=wins[r][:])
                nc.vector.tensor_copy(
                    out=out_sb[:, 2 * rpp + r : 2 * rpp + r + 1], in_=w8[:, 0:1]
                )

            nc.sync.dma_start(out=o_all[:], in_=out_sb[:])

    nc.compile()
    return nc


def make_in_maps(predicted, rows=R, v=V, n_cores=N_CORES):
    """Shard full inputs into per-core in_maps (host-side glue)."""
    rpp = rows // P
    flat = predicted.reshape(rows * n_cores, v).astype(ml_dtypes.bfloat16)
    row_of = np.arange(P)[:, None] * rpp + np.arange(rpp)[None, :]  # [P, rpp]
    rb = (row_of * NB).astype(np.uint32)
    in_maps = []
    for core in range(n_cores):
        in_maps.append(
            {"logits": flat[core * rows : (core + 1) * rows], "rb": rb}
        )
    return in_maps


def combine(results, predicted, target, rows=R, v=V, n_cores=N_CORES):
    """Host-side combine of per-core outputs into the final scalar loss."""
    rpp = rows // P
    n_rows = rows * n_cores

    sumexp = np.empty(n_rows, np.float64)
    am = np.empty(n_rows, np.int64)
    for core in range(n_cores):
        o = results[core]["o_all"].astype(np.float64)  # [P, 3*rpp]
        base = core * rows
        # row (within core) = p*rpp + r  ->  plain C-order reshape of [P, rpp]
        sumexp[base : base + rows] = o[:, 0:rpp].reshape(rows)
        cidx = np.rint(o[:, rpp : 2 * rpp].reshape(rows)).astype(np.int64)
        widx = np.rint(o[:, 2 * rpp : 3 * rpp].reshape(rows)).astype(np.int64)
        am[base : base + rows] = cidx * BW + widx

    tgt = target.reshape(n_rows).astype(np.int64)
    xt = predicted.reshape(n_rows, v)[np.arange(n_rows), tgt].astype(np.float64)
    lse = np.log(sumexp)
    valid = tgt != IGNORE
    nll = lse - xt
    denom = max(float(valid.sum()), 1.0)
    ce = float((nll * valid).sum()) / denom

    am2 = am.reshape(B, S)
    tg2 = tgt.reshape(B, S)

    def first_stop_and_count(ids):
        stop = ids == EOS_ID
        stop[:, -1] = True
        first = np.argmax(stop, axis=1)
        pos_mask = np.arange(ids.shape[1])[None, :] <= first[:, None]
        cnt = np.sum((ids == NEXT_LINE) & pos_mask, axis=1)
        return first, cnt

    lens_p, cnt_p = first_stop_and_count(am2)
    lens_t, cnt_t = first_stop_and_count(tg2)
    len_loss = float(np.mean(np.abs(lens_p - lens_t).astype(np.float64)))
    line_loss = float(np.mean(np.abs(cnt_p - cnt_t).astype(np.float64)))

    loss = ALPHAS[0] * ce + ALPHAS[1] * len_loss + ALPHAS[2] * line_loss
    return np.asarray(loss, dtype=np.float32)


_NC_CACHE = {}


def _get_nc():
    if "nc" not in _NC_CACHE:
        _NC_CACHE["nc"] = build_bass()
    return _NC_CACHE["nc"]


def kernel(predicted, target, _trace=False):
    predicted = np.asarray(predicted, dtype=np.float32)
    target = np.asarray(target, dtype=np.int32)
    nc = _get_nc()
    in_maps = make_in_maps(predicted)
    res = bass_utils.run_bass_kernel_spmd(
        nc, in_maps, core_ids=list(range(N_CORES)), trace=_trace
    )
    out = combine(res.results, predicted, target)
    if _trace:
        return out, res
    return out
